# revision 1
# baseline (speedup 1.0000x reference)
"""Multi-head attention on 8 Trainium2 NeuronCores (tensor-parallel over heads).

B=4, S=2048, D=1024, H=16 heads of DK=64. Each core owns 2 heads (a
128-channel slice of the QKV projections). Per core, per batch b:
  xT   = transpose(x[b])           [d=128 x 8, S]  (DMA transpose, bf16)
  QT   = (Wq_c)^T x^T + bq_c       [128, S]        (channels on partitions)
  KT   = (Wk_c)^T x^T + bk_c       [128, S]
  V    = x Wv_c + bv_c             [S, 128] stored per-head with a ones col
  per head h, per q-pair qp (1024 q cols), accumulate over k-chunks kc:
    scT = K Q^T            [k=128, q=1024] psum (2 matmuls)
    ex  = exp(scT / 8)     bf16 (one wide activation)
    av += V_aug^T ex       [65, 512] x2 psum; rows 0-63 ctx^T, row 64 sumexp
  ctxT = av[0:64] * recip(av[64])  (recip broadcast via DRAM bounce)
  out[b] partial = ctx^T Wo_c      [S, D] fp32  (host sums partials + bo)

Matmul inputs are bf16 (1 cyc/col on PE); accumulation is fp32 in PSUM;
softmax stats and the output are fp32. The emission is software-pipelined:
batch b's attention (ACT-heavy) is interleaved with batch b+1's projections
and batch b-1's output projection (PE/DMA-heavy) so all engines stay fed.
"""

import numpy as np

B, S, D, H, DK = 4, 2048, 1024, 16, 64
NCORES = 8
CS = D // NCORES  # 128 channels (2 heads) per core
NSB = S // 128    # 16 s-blocks
NST = S // 512    # 4 s-tiles
NDC = D // 128    # 8 d-chunks

DTYPE = "bf16"  # "bf16" | "fp32"
TRACE = False
LAST_RESULTS = None
_CACHE = {}


def _interleave(main, fill, start_frac=0.2):
    """Spread fill units evenly between main units (order preserved).
    No fill before start_frac of main has been emitted: the engines run
    in static order, so a fill unit whose inputs aren't ready yet would
    stall them."""
    out = []
    fi = 0
    n0 = int(len(main) * start_frac)
    for i, u in enumerate(main):
        out.append(u)
        if i < n0:
            continue
        want = (i - n0 + 1) * len(fill) // max(1, len(main) - n0)
        while fi < want:
            out.append(fill[fi])
            fi += 1
    out.extend(fill[fi:])
    return out


def _build(repeat=1, bench_io=False, dtype=DTYPE):
    import concourse.bass as bass  # noqa: F401
    import concourse.mybir as mybir
    import concourse.tile as tile
    from concourse import bacc
    from concourse.masks import make_identity

    fp32 = mybir.dt.float32
    cdt = mybir.dt.bfloat16 if dtype == "bf16" else fp32
    AF = mybir.ActivationFunctionType

    nc = bacc.Bacc(None, target_bir_lowering=False)
    if bench_io:
        # timing variant: big tensors stay on-device (garbage contents), tiny
        # external I/O so per-call tunnel transfers don't mask exec time
        x_d = nc.dram_tensor("xint", [B, S, D], cdt)
        out_d = nc.dram_tensor("outint", [B, S, D], fp32)
        xin_d = nc.declare_dram_parameter("xin", [128, 128], fp32, isOutput=False)
        xout_d = nc.declare_dram_parameter("xout", [128, 128], fp32, isOutput=True)
    else:
        x_d = nc.declare_dram_parameter("x", [B, S, D], cdt, isOutput=False)
        out_d = nc.declare_dram_parameter("out", [B, S, D], fp32, isOutput=True)
    wq_d = nc.declare_dram_parameter("wq", [D, CS], cdt, isOutput=False)
    wk_d = nc.declare_dram_parameter("wk", [D, CS], cdt, isOutput=False)
    wv_d = nc.declare_dram_parameter("wv", [D, CS], cdt, isOutput=False)
    wo_d = nc.declare_dram_parameter("wo", [CS, D], cdt, isOutput=False)
    bq_d = nc.declare_dram_parameter("bq", [CS], fp32, isOutput=False)
    bk_d = nc.declare_dram_parameter("bk", [CS], fp32, isOutput=False)
    bv_d = nc.declare_dram_parameter("bv", [CS], fp32, isOutput=False)

    with tile.TileContext(nc) as tc:
        with (
            tc.tile_pool(name="consts", bufs=1) as consts,
            tc.tile_pool(name="xt", bufs=2) as xt_pool,
            tc.tile_pool(name="xload", bufs=3) as xload,
            tc.tile_pool(name="qk", bufs=2) as qk_pool,
            tc.tile_pool(name="vp", bufs=2) as v_pool,
            tc.tile_pool(name="exp", bufs=6) as exp_pool,
            tc.tile_pool(name="ctx", bufs=2) as ctx_pool,
            tc.tile_pool(name="avs", bufs=6) as avs_pool,
            tc.tile_pool(name="rec", bufs=4) as rec_pool,
            tc.tile_pool(name="rb", bufs=4) as rb_pool,
            tc.tile_pool(name="outp", bufs=4) as out_pool,
            tc.tile_pool(name="drp", bufs=8, space="DRAM") as dram_pool,
            tc.tile_pool(name="ps1024", bufs=2, space="PSUM") as ps1024,
            tc.tile_pool(name="ps512", bufs=2, space="PSUM") as ps512,
            tc.tile_pool(name="psav", bufs=1, space="PSUM") as psav,
        ):
            # ---- constants (tiles now, loads deferred until after the
            # first x-transpose DMAs are queued) ----
            wq_t = consts.tile([128, NDC, CS], cdt, tag="wq")
            wk_t = consts.tile([128, NDC, CS], cdt, tag="wk")
            wv_t = consts.tile([128, NDC, CS], cdt, tag="wv")
            wo_t = consts.tile([128, D], cdt, tag="wo")
            bq_t = consts.tile([128, 1], fp32, tag="bq")
            bk_t = consts.tile([128, 1], fp32, tag="bk")
            ones_r = consts.tile([128, 64], fp32, tag="ones_r")
            bv_b = consts.tile([128, CS], fp32, tag="bvb")
            if dtype != "bf16":
                ident = consts.tile([128, 128], cdt, tag="ident")

            def load_consts():
                nc.sync.dma_start(
                    wq_t[:], wq_d[:].rearrange("(c p) m -> p c m", p=128)
                )
                nc.sync.dma_start(
                    wk_t[:], wk_d[:].rearrange("(c p) m -> p c m", p=128)
                )
                nc.sync.dma_start(
                    wv_t[:], wv_d[:].rearrange("(c p) m -> p c m", p=128)
                )
                nc.sync.dma_start(wo_t[:], wo_d[:])
                nc.sync.dma_start(bq_t[:], bq_d[:].rearrange("(p o) -> p o", o=1))
                nc.sync.dma_start(bk_t[:], bk_d[:].rearrange("(p o) -> p o", o=1))
                nc.gpsimd.memset(ones_r[:], 1.0)
                nc.sync.dma_start(
                    bv_b[:],
                    bv_d[:].rearrange("(o f) -> o f", o=1).partition_broadcast(128),
                )
                if dtype != "bf16":
                    make_identity(nc, ident[:])
                if bench_io:
                    tio = consts.tile([128, 128], fp32, tag="tio")
                    nc.sync.dma_start(tio[:], xin_d[:])
                    nc.sync.dma_start(xout_d[:], tio[:])

            state = {}

            def A_units(bi, b):
                """x transpose + QKV projections for batch index bi."""
                xT = xt_pool.tile([128, NDC, S], cdt, tag="xT")
                QT = qk_pool.tile([128, S], cdt, tag="QT")
                KT = qk_pool.tile([128, S], cdt, tag="KT")
                v0 = v_pool.tile([128, NSB, 65], cdt, tag="v0")
                v1 = v_pool.tile([128, NSB, 65], cdt, tag="v1")
                state[bi] = dict(xT=xT, QT=QT, KT=KT, v0=v0, v1=v1)
                units = []
                if dtype == "bf16":
                    xr = x_d[b].rearrange("M (c p) -> M c p", p=128)
                    for cch in range(NDC):
                        units.append(
                            lambda cch=cch: nc.sync.dma_start(
                                xT[:, cch, :], xr[:, cch], transpose=True
                            )
                        )
                else:
                    for sb in range(NSB):
                        def u_x(sb=sb):
                            xl = xload.tile([128, D], cdt, tag="xl")
                            nc.sync.dma_start(
                                xl[:], x_d[b, sb * 128 : (sb + 1) * 128, :]
                            )
                            for cch in range(NDC):
                                pt = ps512.tile([128, 128], fp32, tag="mm512")
                                nc.tensor.transpose(
                                    pt[:], xl[:, cch * 128 : (cch + 1) * 128],
                                    ident[:],
                                )
                                nc.vector.tensor_copy(
                                    xT[:, cch, sb * 128 : (sb + 1) * 128], pt[:]
                                )
                        units.append(u_x)

                for st in range(NST):
                    def u_q(st=st):
                        sl = slice(st * 512, (st + 1) * 512)
                        pq = ps512.tile([128, 512], fp32, tag="mm512")
                        for cch in range(NDC):
                            nc.tensor.matmul(
                                pq[:], wq_t[:, cch, :], xT[:, cch, sl],
                                start=(cch == 0), stop=(cch == NDC - 1),
                            )
                        nc.vector.tensor_scalar_add(QT[:, sl], pq[:], bq_t[:])
                    units.append(u_q)

                    def u_k(st=st):
                        sl = slice(st * 512, (st + 1) * 512)
                        pk = ps512.tile([128, 512], fp32, tag="mm512")
                        for cch in range(NDC):
                            nc.tensor.matmul(
                                pk[:], wk_t[:, cch, :], xT[:, cch, sl],
                                start=(cch == 0), stop=(cch == NDC - 1),
                            )
                        nc.vector.tensor_scalar_add(KT[:, sl], pk[:], bk_t[:])
                    units.append(u_k)

                def u_ones():
                    nc.gpsimd.memset(v0[:, :, 64:65], 1.0)
                    nc.gpsimd.memset(v1[:, :, 64:65], 1.0)
                units.append(u_ones)

                for sb in range(NSB):
                    def u_v(sb=sb):
                        pv = ps512.tile([128, 128], fp32, tag="mm512")
                        for cch in range(NDC):
                            nc.tensor.matmul(
                                pv[:], xT[:, cch, sb * 128 : (sb + 1) * 128],
                                wv_t[:, cch, :],
                                start=(cch == 0), stop=(cch == NDC - 1),
                            )
                        nc.vector.tensor_add(
                            v0[:, sb, 0:64], pv[:, 0:64], bv_b[:, 0:64]
                        )
                        nc.vector.tensor_add(
                            v1[:, sb, 0:64], pv[:, 64:128], bv_b[:, 64:128]
                        )
                    units.append(u_v)
                return units

            def B_units(bi):
                st_ = state[bi]
                QT, KT, v0, v1 = st_["QT"], st_["KT"], st_["v0"], st_["v1"]
                ctx0 = ctx_pool.tile([128, S // 2], cdt, tag="ctx0")
                ctx1 = ctx_pool.tile([128, S // 2], cdt, tag="ctx1")
                st_["ctx"] = (ctx0, ctx1)
                sections = []
                for qp in range(NST // 2):
                    q0 = qp * 1024
                    ctxq = (ctx0, ctx1)[qp]
                    units = []
                    for h in (0, 1):
                        hoff = 64 * h
                        vh = v0 if h == 0 else v1
                        avpair = []

                        def u_alloc(avpair=avpair):
                            av0_t = psav.tile([65, 512], fp32, tag="av0")
                            avpair.append(av0_t)
                            av1_t = psav.tile([65, 512], fp32, tag="av1")
                            avpair.append(av1_t)
                        units.append(u_alloc)

                        exq = []

                        def u_sc(hoff=hoff, q0=q0, kc=0, exq=exq):
                            ksl = slice(kc * 128, (kc + 1) * 128)
                            sc = ps1024.tile([128, 1024], fp32, tag="mm1024")
                            nc.tensor.matmul(
                                sc[:, 0:512],
                                KT[hoff : hoff + 64, ksl],
                                QT[hoff : hoff + 64, q0 : q0 + 512],
                                start=True, stop=True,
                            )
                            nc.tensor.matmul(
                                sc[:, 512:1024],
                                KT[hoff : hoff + 64, ksl],
                                QT[hoff : hoff + 64, q0 + 512 : q0 + 1024],
                                start=True, stop=True,
                            )
                            ex = exp_pool.tile([128, 1024], cdt, tag="ex")
                            nc.scalar.activation(ex[:], sc[:], AF.Exp, scale=0.125)
                            exq.append(ex)

                        def u_av(vh=vh, kc=0, avpair=avpair, exq=exq):
                            ex = exq[kc]
                            for sub in (0, 1):
                                nc.tensor.matmul(
                                    avpair[sub][:],
                                    vh[:, kc, :],
                                    ex[:, sub * 512 : (sub + 1) * 512],
                                    start=(kc == 0), stop=(kc == NSB - 1),
                                    skip_group_check=True,
                                )

                        from functools import partial
                        for kc in range(NSB):
                            def u_kc(kc=kc, u_sc=u_sc, u_av=u_av):
                                u_sc(kc=kc)
                                if kc > 0:
                                    u_av(kc=kc - 1)
                                if kc == NSB - 1:
                                    u_av(kc=kc)
                            units.append(u_kc)

                        def u_norm(hoff=hoff, ctxq=ctxq, avpair=avpair):
                            for sub in (0, 1):
                                avx = avpair[sub]
                                qsl = slice(sub * 512, (sub + 1) * 512)
                                # free the psum bank fast, then normalize
                                av_s = avs_pool.tile([65, 512], fp32, tag="avs")
                                nc.vector.tensor_copy(av_s[:], avx[:])
                                rec = rec_pool.tile([65, 512], fp32, tag="rec")
                                nc.vector.reciprocal(rec[64:65, :], av_s[64:65, :])
                                dr = dram_pool.tile([1, 512], fp32, tag="dr")
                                nc.sync.dma_start(dr[:], rec[64:65, :])
                                rb = rb_pool.tile([64, 512], fp32, tag="rb")
                                nc.sync.dma_start(rb[:], dr[:].partition_broadcast(64))
                                nc.vector.tensor_mul(
                                    ctxq[hoff : hoff + 64, qsl], av_s[0:64, :], rb[:]
                                )
                        units.append(u_norm)
                    sections.append(units)
                return sections

            def C_units(bi, b):
                st_ = state[bi]
                ctx = st_["ctx"]
                halves = ([], [])
                for sb in range(NSB):
                    def u_o(sb=sb):
                        ctxq = ctx[sb // 8]
                        lsl = slice((sb % 8) * 128, (sb % 8 + 1) * 128)
                        ot = out_pool.tile([128, D], fp32, tag="ot")
                        for half in range(2):
                            osl = slice(half * 512, (half + 1) * 512)
                            po = ps512.tile([128, 512], fp32, tag="mm512")
                            nc.tensor.matmul(
                                po[:], ctxq[:, lsl], wo_t[:, osl],
                                start=True, stop=True,
                            )
                            nc.vector.tensor_copy(ot[:, osl], po[:])
                        nc.sync.dma_start(
                            out_d[b, sb * 128 : (sb + 1) * 128, :], ot[:]
                        )
                    halves[sb // 8].append(u_o)
                return halves

            bs = [bb for _ in range(repeat) for bb in range(B)]
            n_x_units = NDC if dtype == "bf16" else NSB
            a0 = A_units(0, bs[0])
            for u in a0[:n_x_units]:
                u()
            load_consts()
            for u in a0[n_x_units:]:
                u()
            c_tail = []
            for bi, b in enumerate(bs):
                sec0, sec1 = B_units(bi)
                if bi + 1 < len(bs):
                    a_next = A_units(bi + 1, bs[bi + 1])
                    xdmas, a_rest = a_next[:n_x_units], a_next[n_x_units:]
                else:
                    xdmas, a_rest = [], []
                # start next batch's x transposes immediately
                for u in xdmas:
                    u()
                half = len(a_rest) // 2
                for u in _interleave(sec0, c_tail + a_rest[:half]):
                    u()
                c_head, new_tail = C_units(bi, b)
                for u in _interleave(sec1, c_head + a_rest[half:]):
                    u()
                c_tail = new_tail
                del state[bi]
            for u in c_tail:
                u()

    nc.compile()
    return nc


def _get_nc(repeat=1, bench_io=False, dtype=None):
    if dtype is None:
        dtype = DTYPE
    key = f"nc{repeat}_{bench_io}_{dtype}"
    if key not in _CACHE:
        _CACHE[key] = _build(repeat, bench_io, dtype)
    return _CACHE[key]


def kernel(**inputs):
    global LAST_RESULTS
    import ml_dtypes
    from concourse.bass_utils import run_bass_kernel_spmd

    cdt = ml_dtypes.bfloat16 if DTYPE == "bf16" else np.float32
    x = np.ascontiguousarray(np.asarray(inputs["x"], dtype=np.float32).astype(cdt))
    Wq = np.asarray(inputs["Wq"], dtype=np.float32).astype(cdt)
    Wk = np.asarray(inputs["Wk"], dtype=np.float32).astype(cdt)
    Wv = np.asarray(inputs["Wv"], dtype=np.float32).astype(cdt)
    Wo = np.asarray(inputs["Wo"], dtype=np.float32).astype(cdt)
    bq = np.asarray(inputs["bq"], dtype=np.float32)
    bk = np.asarray(inputs["bk"], dtype=np.float32)
    bv = np.asarray(inputs["bv"], dtype=np.float32)
    bo = np.asarray(inputs["bo"], dtype=np.float32)

    nc = _get_nc()
    in_maps = []
    for c in range(NCORES):
        cs = slice(CS * c, CS * (c + 1))
        in_maps.append(
            {
                "x": x,
                "wq": np.ascontiguousarray(Wq[:, cs]),
                "wk": np.ascontiguousarray(Wk[:, cs]),
                "wv": np.ascontiguousarray(Wv[:, cs]),
                "wo": np.ascontiguousarray(Wo[cs, :]),
                "bq": np.ascontiguousarray(bq[cs]),
                "bk": np.ascontiguousarray(bk[cs]),
                "bv": np.ascontiguousarray(bv[cs]),
            }
        )
    res = run_bass_kernel_spmd(
        nc, in_maps, core_ids=list(range(NCORES)), trace=TRACE
    )
    LAST_RESULTS = res
    acc = np.zeros((B, S, D), dtype=np.float64)
    for c in range(NCORES):
        acc += res.results[c]["out"]
    acc += bo
    return acc.astype(np.float32)



# revision 6
# speedup vs baseline: 1.1169x; 1.1169x over previous
"""Multi-head attention on 8 Trainium2 NeuronCores (tensor-parallel over heads).

B=4, S=2048, D=1024, H=16 heads of DK=64. Each core owns 2 heads (a
128-channel slice of the QKV projections). Per core, per batch b:
  xT   = transpose(x[b])           [d=128 x 8, S]  (DMA transpose, bf16)
  QT   = (Wq_c)^T x^T + bq_c       [128, S]        (channels on partitions)
  KT   = (Wk_c)^T x^T + bk_c       [128, S]
  V    = x Wv_c + bv_c             [S, 128] stored per-head with a ones col
  per q-tile (512 q cols), accumulate over k-chunks kc:
    scT = K Q^T            [k=128, 2x512] psum; both heads issued
                           back-to-back as concurrent PE row tiles
                           (rows 0-63 / 64-127) sharing one QT stream
    ex  = exp(scT / 8)     bf16 (one wide activation, both heads)
    av_h += V_aug^T ex_h   [65, 512] per head; rows 0-63 ctx^T, row 64 sumexp
  ctxT = av[0:64] * recip(av[64])  (recip on the 64-row broadcast)
  out[b] partial = ctx^T Wo_c      [S, D] fp32  (host sums partials + bo)

Matmul inputs are bf16 (1 cyc/col on PE); accumulation is fp32 in PSUM;
softmax stats and the output are fp32. The emission is software-pipelined:
batch b's attention (ACT-heavy) is interleaved with batch b+1's projections
and batch b-1's output projection (PE/DMA-heavy) so all engines stay fed.
"""

import numpy as np

B, S, D, H, DK = 4, 2048, 1024, 16, 64
NCORES = 8
CS = D // NCORES  # 128 channels (2 heads) per core
NSB = S // 128    # 16 s-blocks
NST = S // 512    # 4 s-tiles
NDC = D // 128    # 8 d-chunks

DTYPE = "bf16"  # "bf16" | "fp32"
TRACE = False
LAST_RESULTS = None
_CACHE = {}


def _interleave(main, fill, start_frac=0.2):
    """Spread fill units evenly between main units (order preserved).
    No fill before start_frac of main has been emitted: the engines run
    in static order, so a fill unit whose inputs aren't ready yet would
    stall them."""
    out = []
    fi = 0
    n0 = int(len(main) * start_frac)
    for i, u in enumerate(main):
        out.append(u)
        if i < n0:
            continue
        want = (i - n0 + 1) * len(fill) // max(1, len(main) - n0)
        while fi < want:
            out.append(fill[fi])
            fi += 1
    out.extend(fill[fi:])
    return out


def _build(repeat=1, bench_io=False, dtype=DTYPE):
    import concourse.bass as bass  # noqa: F401
    import concourse.mybir as mybir
    import concourse.tile as tile
    from concourse import bacc
    from concourse.masks import make_identity

    fp32 = mybir.dt.float32
    cdt = mybir.dt.bfloat16 if dtype == "bf16" else fp32
    AF = mybir.ActivationFunctionType

    nc = bacc.Bacc(None, target_bir_lowering=False)
    if bench_io:
        # timing variant: big tensors stay on-device (garbage contents), tiny
        # external I/O so per-call tunnel transfers don't mask exec time
        x_d = nc.dram_tensor("xint", [B, S, D], cdt)
        out_d = nc.dram_tensor("outint", [B, S, D], fp32)
        xin_d = nc.declare_dram_parameter("xin", [128, 128], fp32, isOutput=False)
        xout_d = nc.declare_dram_parameter("xout", [128, 128], fp32, isOutput=True)
    else:
        x_d = nc.declare_dram_parameter("x", [B, S, D], cdt, isOutput=False)
        out_d = nc.declare_dram_parameter("out", [B, S, D], fp32, isOutput=True)
    wq_d = nc.declare_dram_parameter("wq", [D, CS], cdt, isOutput=False)
    wk_d = nc.declare_dram_parameter("wk", [D, CS], cdt, isOutput=False)
    wv_d = nc.declare_dram_parameter("wv", [D, CS], cdt, isOutput=False)
    wo_d = nc.declare_dram_parameter("wo", [CS, D], cdt, isOutput=False)
    bq_d = nc.declare_dram_parameter("bq", [CS], fp32, isOutput=False)
    bk_d = nc.declare_dram_parameter("bk", [CS], fp32, isOutput=False)
    bv_d = nc.declare_dram_parameter("bv", [CS], fp32, isOutput=False)

    with tile.TileContext(nc) as tc:
        with (
            tc.tile_pool(name="consts", bufs=1) as consts,
            tc.tile_pool(name="xt", bufs=2) as xt_pool,
            tc.tile_pool(name="xload", bufs=3) as xload,
            tc.tile_pool(name="qk", bufs=2) as qk_pool,
            tc.tile_pool(name="vp", bufs=2) as v_pool,
            tc.tile_pool(name="exp", bufs=6) as exp_pool,
            tc.tile_pool(name="ctx", bufs=2) as ctx_pool,
            tc.tile_pool(name="avs", bufs=6) as avs_pool,
            tc.tile_pool(name="rec", bufs=4) as rec_pool,
            tc.tile_pool(name="rb", bufs=4) as rb_pool,
            tc.tile_pool(name="outp", bufs=4) as out_pool,
            tc.tile_pool(name="drp", bufs=8, space="DRAM") as dram_pool,
            tc.tile_pool(name="ps1024", bufs=2, space="PSUM") as ps1024,
            tc.tile_pool(name="ps512", bufs=2, space="PSUM") as ps512,
            tc.tile_pool(name="psav", bufs=1, space="PSUM") as psav,
        ):
            # ---- constants (tiles now, loads deferred until after the
            # first x-transpose DMAs are queued) ----
            wq_t = consts.tile([128, NDC, CS], cdt, tag="wq")
            wk_t = consts.tile([128, NDC, CS], cdt, tag="wk")
            wv_t = consts.tile([128, NDC, CS], cdt, tag="wv")
            wo_t = consts.tile([128, D], cdt, tag="wo")
            bq_t = consts.tile([128, 1], fp32, tag="bq")
            bk_t = consts.tile([128, 1], fp32, tag="bk")
            ones_r = consts.tile([128, 64], fp32, tag="ones_r")
            bv_b = consts.tile([128, CS], fp32, tag="bvb")
            if dtype != "bf16":
                ident = consts.tile([128, 128], cdt, tag="ident")

            def load_consts():
                nc.sync.dma_start(
                    wq_t[:], wq_d[:].rearrange("(c p) m -> p c m", p=128)
                )
                nc.sync.dma_start(
                    wk_t[:], wk_d[:].rearrange("(c p) m -> p c m", p=128)
                )
                nc.sync.dma_start(
                    wv_t[:], wv_d[:].rearrange("(c p) m -> p c m", p=128)
                )
                nc.sync.dma_start(wo_t[:], wo_d[:])
                nc.sync.dma_start(bq_t[:], bq_d[:].rearrange("(p o) -> p o", o=1))
                nc.sync.dma_start(bk_t[:], bk_d[:].rearrange("(p o) -> p o", o=1))
                nc.gpsimd.memset(ones_r[:], 1.0)
                nc.sync.dma_start(
                    bv_b[:],
                    bv_d[:].rearrange("(o f) -> o f", o=1).partition_broadcast(128),
                )
                if dtype != "bf16":
                    make_identity(nc, ident[:])
                if bench_io:
                    tio = consts.tile([128, 128], fp32, tag="tio")
                    nc.sync.dma_start(tio[:], xin_d[:])
                    nc.sync.dma_start(xout_d[:], tio[:])

            state = {}

            def A_units(bi, b):
                """x transpose + QKV projections for batch index bi."""
                xT = xt_pool.tile([128, NDC, S], cdt, tag="xT")
                QT = qk_pool.tile([128, S], cdt, tag="QT")
                KT = qk_pool.tile([128, S], cdt, tag="KT")
                v0 = v_pool.tile([128, NSB, 65], cdt, tag="v0")
                v1 = v_pool.tile([128, NSB, 65], cdt, tag="v1")
                state[bi] = dict(xT=xT, QT=QT, KT=KT, v0=v0, v1=v1)
                units = []
                if dtype == "bf16":
                    xr = x_d[b].rearrange("M (c p) -> M c p", p=128)
                    for cch in range(NDC):
                        units.append(
                            lambda cch=cch: nc.sync.dma_start(
                                xT[:, cch, :], xr[:, cch], transpose=True
                            )
                        )
                else:
                    for sb in range(NSB):
                        def u_x(sb=sb):
                            xl = xload.tile([128, D], cdt, tag="xl")
                            nc.sync.dma_start(
                                xl[:], x_d[b, sb * 128 : (sb + 1) * 128, :]
                            )
                            for cch in range(NDC):
                                pt = ps512.tile([128, 128], fp32, tag="mm512")
                                nc.tensor.transpose(
                                    pt[:], xl[:, cch * 128 : (cch + 1) * 128],
                                    ident[:],
                                )
                                nc.vector.tensor_copy(
                                    xT[:, cch, sb * 128 : (sb + 1) * 128], pt[:]
                                )
                        units.append(u_x)

                for st in range(NST):
                    def u_q(st=st):
                        sl = slice(st * 512, (st + 1) * 512)
                        pq = ps512.tile([128, 512], fp32, tag="mm512")
                        for cch in range(NDC):
                            nc.tensor.matmul(
                                pq[:], wq_t[:, cch, :], xT[:, cch, sl],
                                start=(cch == 0), stop=(cch == NDC - 1),
                            )
                        nc.vector.tensor_scalar_add(QT[:, sl], pq[:], bq_t[:])
                    units.append(u_q)

                    def u_k(st=st):
                        sl = slice(st * 512, (st + 1) * 512)
                        pk = ps512.tile([128, 512], fp32, tag="mm512")
                        for cch in range(NDC):
                            nc.tensor.matmul(
                                pk[:], wk_t[:, cch, :], xT[:, cch, sl],
                                start=(cch == 0), stop=(cch == NDC - 1),
                            )
                        nc.vector.tensor_scalar_add(KT[:, sl], pk[:], bk_t[:])
                    units.append(u_k)

                def u_ones():
                    nc.gpsimd.memset(v0[:, :, 64:65], 1.0)
                    nc.gpsimd.memset(v1[:, :, 64:65], 1.0)
                units.append(u_ones)

                for sb in range(NSB):
                    def u_v(sb=sb):
                        pv = ps512.tile([128, 128], fp32, tag="mm512")
                        for cch in range(NDC):
                            nc.tensor.matmul(
                                pv[:], xT[:, cch, sb * 128 : (sb + 1) * 128],
                                wv_t[:, cch, :],
                                start=(cch == 0), stop=(cch == NDC - 1),
                            )
                        nc.vector.tensor_add(
                            v0[:, sb, 0:64], pv[:, 0:64], bv_b[:, 0:64]
                        )
                        nc.vector.tensor_add(
                            v1[:, sb, 0:64], pv[:, 64:128], bv_b[:, 64:128]
                        )
                    units.append(u_v)
                return units

            def B_units(bi):
                """Attention for batch bi, one section per 512-wide q-tile.

                Per kc the two heads' score matmuls are issued back-to-back:
                their stationaries sit in disjoint row groups (partitions
                0-63 / 64-127), so the PE runs them as concurrent row tiles
                fed by one shared QT stream, writing the two halves of one
                [128, 1024] psum pair-tile. One wide exp covers both heads.
                """
                st_ = state[bi]
                QT, KT, v0, v1 = st_["QT"], st_["KT"], st_["v0"], st_["v1"]
                ctx0 = ctx_pool.tile([128, S // 2], cdt, tag="ctx0")
                ctx1 = ctx_pool.tile([128, S // 2], cdt, tag="ctx1")
                st_["ctx"] = (ctx0, ctx1)
                sections = []
                for qt in range(NST):
                    q0 = qt * 512
                    ctxq = (ctx0, ctx1)[qt // 2]
                    qsl_ctx = slice((qt % 2) * 512, (qt % 2 + 1) * 512)
                    units = []
                    avh = []

                    def u_alloc(avh=avh):
                        av0_t = psav.tile([65, 512], fp32, tag="av0")
                        avh.append(av0_t)
                        av1_t = psav.tile([65, 512], fp32, tag="av1")
                        avh.append(av1_t)
                    units.append(u_alloc)

                    exq = []

                    def u_sc(kc, q0=q0, exq=exq):
                        ksl = slice(kc * 128, (kc + 1) * 128)
                        sc = ps1024.tile([128, 1024], fp32, tag="mm1024")
                        nc.tensor.matmul(
                            sc[:, 0:512],
                            KT[0:64, ksl],
                            QT[0:64, q0 : q0 + 512],
                            start=True, stop=True,
                        )
                        nc.tensor.matmul(
                            sc[:, 512:1024],
                            KT[64:128, ksl],
                            QT[64:128, q0 : q0 + 512],
                            start=True, stop=True,
                        )
                        ex = exp_pool.tile([128, 1024], cdt, tag="ex")
                        nc.scalar.activation(ex[:], sc[:], AF.Exp, scale=0.125)
                        exq.append(ex)

                    def u_av(kc, avh=avh, exq=exq):
                        ex = exq[kc]
                        nc.tensor.matmul(
                            avh[0][:], v0[:, kc, :], ex[:, 0:512],
                            start=(kc == 0), stop=(kc == NSB - 1),
                            skip_group_check=True,
                        )
                        nc.tensor.matmul(
                            avh[1][:], v1[:, kc, :], ex[:, 512:1024],
                            start=(kc == 0), stop=(kc == NSB - 1),
                            skip_group_check=True,
                        )

                    for kc in range(NSB):
                        def u_kc(kc=kc, u_sc=u_sc, u_av=u_av):
                            u_sc(kc)
                            if kc > 0:
                                u_av(kc - 1)
                            if kc == NSB - 1:
                                u_av(kc)
                        units.append(u_kc)

                    def u_norm(ctxq=ctxq, qsl_ctx=qsl_ctx, avh=avh):
                        for h in (0, 1):
                            # free the psum bank fast, then normalize:
                            # reciprocal runs on the 64-row broadcast (64
                            # lanes) instead of the 1-row sumexp.
                            av_s = avs_pool.tile([65, 512], fp32, tag="avs")
                            nc.vector.tensor_copy(av_s[:], avh[h][:])
                            dr = dram_pool.tile([1, 512], fp32, tag="dr")
                            nc.sync.dma_start(dr[:], av_s[64:65, :])
                            rb = rb_pool.tile([64, 512], fp32, tag="rb")
                            nc.sync.dma_start(rb[:], dr[:].partition_broadcast(64))
                            rec = rec_pool.tile([64, 512], fp32, tag="rec")
                            nc.vector.reciprocal(rec[:], rb[:])
                            nc.vector.tensor_mul(
                                ctxq[64 * h : 64 * h + 64, qsl_ctx],
                                av_s[0:64, :], rec[:],
                            )
                    units.append(u_norm)
                    sections.append(units)
                return sections

            def C_units(bi, b):
                st_ = state[bi]
                ctx = st_["ctx"]
                halves = ([], [])
                for sb in range(NSB):
                    def u_o(sb=sb):
                        ctxq = ctx[sb // 8]
                        lsl = slice((sb % 8) * 128, (sb % 8 + 1) * 128)
                        ot = out_pool.tile([128, D], fp32, tag="ot")
                        for half in range(2):
                            osl = slice(half * 512, (half + 1) * 512)
                            po = ps512.tile([128, 512], fp32, tag="mm512")
                            nc.tensor.matmul(
                                po[:], ctxq[:, lsl], wo_t[:, osl],
                                start=True, stop=True,
                            )
                            nc.vector.tensor_copy(ot[:, osl], po[:])
                        nc.sync.dma_start(
                            out_d[b, sb * 128 : (sb + 1) * 128, :], ot[:]
                        )
                    halves[sb // 8].append(u_o)
                return halves

            bs = [bb for _ in range(repeat) for bb in range(B)]
            n_x_units = NDC if dtype == "bf16" else NSB
            a0 = A_units(0, bs[0])
            for u in a0[:n_x_units]:
                u()
            load_consts()
            for u in a0[n_x_units:]:
                u()
            c_tail = []
            for bi, b in enumerate(bs):
                secs = B_units(bi)
                if bi + 1 < len(bs):
                    a_next = A_units(bi + 1, bs[bi + 1])
                    xdmas, a_rest = a_next[:n_x_units], a_next[n_x_units:]
                else:
                    xdmas, a_rest = [], []
                # start next batch's x transposes immediately
                for u in xdmas:
                    u()
                half = len(a_rest) // 2
                sec01 = secs[0] + secs[1]
                sec23 = secs[2] + secs[3]
                for u in _interleave(sec01, c_tail + a_rest[:half]):
                    u()
                c_head, new_tail = C_units(bi, b)
                for u in _interleave(sec23, c_head + a_rest[half:]):
                    u()
                c_tail = new_tail
                del state[bi]
            for u in c_tail:
                u()

    nc.compile()
    return nc


def _get_nc(repeat=1, bench_io=False, dtype=None):
    if dtype is None:
        dtype = DTYPE
    key = f"nc{repeat}_{bench_io}_{dtype}"
    if key not in _CACHE:
        _CACHE[key] = _build(repeat, bench_io, dtype)
    return _CACHE[key]


def kernel(**inputs):
    global LAST_RESULTS
    import ml_dtypes
    from concourse.bass_utils import run_bass_kernel_spmd

    cdt = ml_dtypes.bfloat16 if DTYPE == "bf16" else np.float32
    x = np.ascontiguousarray(np.asarray(inputs["x"], dtype=np.float32).astype(cdt))
    Wq = np.asarray(inputs["Wq"], dtype=np.float32).astype(cdt)
    Wk = np.asarray(inputs["Wk"], dtype=np.float32).astype(cdt)
    Wv = np.asarray(inputs["Wv"], dtype=np.float32).astype(cdt)
    Wo = np.asarray(inputs["Wo"], dtype=np.float32).astype(cdt)
    bq = np.asarray(inputs["bq"], dtype=np.float32)
    bk = np.asarray(inputs["bk"], dtype=np.float32)
    bv = np.asarray(inputs["bv"], dtype=np.float32)
    bo = np.asarray(inputs["bo"], dtype=np.float32)

    nc = _get_nc()
    in_maps = []
    for c in range(NCORES):
        cs = slice(CS * c, CS * (c + 1))
        in_maps.append(
            {
                "x": x,
                "wq": np.ascontiguousarray(Wq[:, cs]),
                "wk": np.ascontiguousarray(Wk[:, cs]),
                "wv": np.ascontiguousarray(Wv[:, cs]),
                "wo": np.ascontiguousarray(Wo[cs, :]),
                "bq": np.ascontiguousarray(bq[cs]),
                "bk": np.ascontiguousarray(bk[cs]),
                "bv": np.ascontiguousarray(bv[cs]),
            }
        )
    res = run_bass_kernel_spmd(
        nc, in_maps, core_ids=list(range(NCORES)), trace=TRACE
    )
    LAST_RESULTS = res
    acc = np.zeros((B, S, D), dtype=np.float64)
    for c in range(NCORES):
        acc += res.results[c]["out"]
    acc += bo
    return acc.astype(np.float32)



# revision 14
# speedup vs baseline: 1.3320x; 1.1926x over previous
"""Multi-head attention on 8 Trainium2 NeuronCores (tensor-parallel over heads).

B=4, S=2048, D=1024, H=16 heads of DK=64. Each core owns 2 heads (a
128-channel slice of the QKV projections). Per core, per batch b:
  xT   = transpose(x[b])           [d=128 x 8, S]  (DMA transpose, bf16)
  QT   = (Wq_c)^T x^T + bq_c       [128, S]        (channels on partitions)
  KT   = (Wk_c)^T x^T + bk_c       [128, S]
  V    = x Wv_c + bv_c             [S, 128] stored per-head with a ones col
  per q-tile (512 q cols), accumulate over k-chunks kc:
    scT = K Q^T            [k=128, 2x512] psum; both heads issued
                           back-to-back as concurrent PE row tiles
                           (rows 0-63 / 64-127) sharing one QT stream
    ex  = exp(scT / 8)     bf16 (one wide activation, both heads)
    av_h += V_aug^T ex_h   [65, 512] per head; rows 0-63 ctx^T, row 64 sumexp
  ctxT = av[0:64] * recip(av[64])  (recip on the 64-row broadcast)
  out[b] partial = ctx^T Wo_c      [S, D] fp32  (host sums partials + bo)

Matmul inputs are bf16 (1 cyc/col on PE); accumulation is fp32 in PSUM;
softmax stats and the output are fp32. The emission is software-pipelined:
batch b's attention (ACT-heavy) is interleaved with batch b+1's projections
and batch b-1's output projection (PE/DMA-heavy) so all engines stay fed.
"""

import numpy as np

B, S, D, H, DK = 4, 2048, 1024, 16, 64
NCORES = 8
CS = D // NCORES  # 128 channels (2 heads) per core
NSB = S // 128    # 16 s-blocks
NST = S // 512    # 4 s-tiles
NDC = D // 128    # 8 d-chunks

DTYPE = "bf16"  # "bf16" | "fp32"
TRACE = False
LAST_RESULTS = None
_CACHE = {}


def _interleave(main, fill, start_frac=0.2):
    """Spread fill units evenly between main units (order preserved).
    No fill before start_frac of main has been emitted: the engines run
    in static order, so a fill unit whose inputs aren't ready yet would
    stall them."""
    out = []
    fi = 0
    n0 = int(len(main) * start_frac)
    for i, u in enumerate(main):
        out.append(u)
        if i < n0:
            continue
        want = (i - n0 + 1) * len(fill) // max(1, len(main) - n0)
        while fi < want:
            out.append(fill[fi])
            fi += 1
    out.extend(fill[fi:])
    return out


def _build(repeat=1, bench_io=False, dtype=DTYPE):
    import concourse.bass as bass  # noqa: F401
    import concourse.mybir as mybir
    import concourse.tile as tile
    from concourse import bacc
    from concourse.masks import make_identity

    fp32 = mybir.dt.float32
    fp16 = mybir.dt.float16
    cdt = mybir.dt.bfloat16 if dtype == "bf16" else fp32
    AF = mybir.ActivationFunctionType

    nc = bacc.Bacc(None, target_bir_lowering=False)
    if bench_io:
        # timing variant: big tensors stay on-device (garbage contents), tiny
        # external I/O so per-call tunnel transfers don't mask exec time
        x_d = nc.dram_tensor("xint", [B, S, D], cdt)
        out_d = nc.dram_tensor("outint", [B, S, D], fp32)
        xin_d = nc.declare_dram_parameter("xin", [128, 128], fp32, isOutput=False)
        xout_d = nc.declare_dram_parameter("xout", [128, 128], fp32, isOutput=True)
    else:
        x_d = nc.declare_dram_parameter("x", [B, S, D], cdt, isOutput=False)
        out_d = nc.declare_dram_parameter("out", [B, S, D], fp32, isOutput=True)
    wq_d = nc.declare_dram_parameter("wq", [D, CS], cdt, isOutput=False)
    wk_d = nc.declare_dram_parameter("wk", [D, CS], cdt, isOutput=False)
    wv_d = nc.declare_dram_parameter("wv", [D, CS], cdt, isOutput=False)
    wo_d = nc.declare_dram_parameter("wo", [CS, D], cdt, isOutput=False)
    bq_d = nc.declare_dram_parameter("bq", [CS], fp32, isOutput=False)
    bk_d = nc.declare_dram_parameter("bk", [CS], fp32, isOutput=False)
    bv_d = nc.declare_dram_parameter("bv", [CS], fp32, isOutput=False)

    with tile.TileContext(nc) as tc:
        with (
            tc.tile_pool(name="consts", bufs=1) as consts,
            tc.tile_pool(name="xt", bufs=2) as xt_pool,
            tc.tile_pool(name="xload", bufs=3) as xload,
            tc.tile_pool(name="qk", bufs=2) as qk_pool,
            tc.tile_pool(name="vp", bufs=2) as v_pool,
            tc.tile_pool(name="exp", bufs=6) as exp_pool,
            tc.tile_pool(name="ctx", bufs=2) as ctx_pool,
            tc.tile_pool(name="avs", bufs=6) as avs_pool,
            tc.tile_pool(name="rec", bufs=4) as rec_pool,
            tc.tile_pool(name="rb", bufs=4) as rb_pool,
            tc.tile_pool(name="outp", bufs=4) as out_pool,
            tc.tile_pool(name="drp", bufs=8, space="DRAM") as dram_pool,
            tc.tile_pool(name="ps1024", bufs=2, space="PSUM") as ps1024,
            tc.tile_pool(name="ps512", bufs=2, space="PSUM") as ps512,
            tc.tile_pool(name="psav", bufs=1, space="PSUM") as psav,
        ):
            # ---- constants (tiles now, loads deferred until after the
            # first x-transpose DMAs are queued) ----
            wq_t = consts.tile([128, NDC, CS], cdt, tag="wq")
            wk_t = consts.tile([128, NDC, CS], cdt, tag="wk")
            wv_t = consts.tile([128, NDC, CS], cdt, tag="wv")
            wo_t = consts.tile([128, D], cdt, tag="wo")
            bq_t = consts.tile([128, 1], fp32, tag="bq")
            bk_t = consts.tile([128, 1], fp32, tag="bk")
            ones_r = consts.tile([128, 64], fp32, tag="ones_r")
            bv_b = consts.tile([128, CS], fp32, tag="bvb")
            if dtype != "bf16":
                ident = consts.tile([128, 128], cdt, tag="ident")

            def load_consts():
                nc.sync.dma_start(
                    wq_t[:], wq_d[:].rearrange("(c p) m -> p c m", p=128)
                )
                nc.sync.dma_start(
                    wk_t[:], wk_d[:].rearrange("(c p) m -> p c m", p=128)
                )
                nc.sync.dma_start(
                    wv_t[:], wv_d[:].rearrange("(c p) m -> p c m", p=128)
                )
                nc.sync.dma_start(wo_t[:], wo_d[:])
                nc.sync.dma_start(bq_t[:], bq_d[:].rearrange("(p o) -> p o", o=1))
                nc.sync.dma_start(bk_t[:], bk_d[:].rearrange("(p o) -> p o", o=1))
                nc.gpsimd.memset(ones_r[:], 1.0)
                nc.sync.dma_start(
                    bv_b[:],
                    bv_d[:].rearrange("(o f) -> o f", o=1).partition_broadcast(128),
                )
                if dtype != "bf16":
                    make_identity(nc, ident[:])
                if bench_io:
                    tio = consts.tile([128, 128], fp32, tag="tio")
                    nc.sync.dma_start(tio[:], xin_d[:])
                    nc.sync.dma_start(xout_d[:], tio[:])

            state = {}

            def A_units(bi, b):
                """x transpose + QKV projections for batch index bi."""
                xT = xt_pool.tile([128, NDC, S], cdt, tag="xT")
                QT = qk_pool.tile([128, S], cdt, tag="QT")
                KT = qk_pool.tile([128, S], cdt, tag="KT")
                v0 = v_pool.tile([128, NSB, 65], cdt, tag="v0")
                v1 = v_pool.tile([128, NSB, 65], cdt, tag="v1")
                state[bi] = dict(xT=xT, QT=QT, KT=KT, v0=v0, v1=v1)
                units = []
                if dtype == "bf16":
                    xr = x_d[b].rearrange("M (c p) -> M c p", p=128)
                    for cch in range(NDC):
                        units.append(
                            lambda cch=cch: nc.sync.dma_start(
                                xT[:, cch, :], xr[:, cch], transpose=True
                            )
                        )
                else:
                    for sb in range(NSB):
                        def u_x(sb=sb):
                            xl = xload.tile([128, D], cdt, tag="xl")
                            nc.sync.dma_start(
                                xl[:], x_d[b, sb * 128 : (sb + 1) * 128, :]
                            )
                            for cch in range(NDC):
                                pt = ps512.tile([128, 128], fp32, tag="mm512")
                                nc.tensor.transpose(
                                    pt[:], xl[:, cch * 128 : (cch + 1) * 128],
                                    ident[:],
                                )
                                nc.vector.tensor_copy(
                                    xT[:, cch, sb * 128 : (sb + 1) * 128], pt[:]
                                )
                        units.append(u_x)

                for st in range(NST):
                    def u_q(st=st):
                        sl = slice(st * 512, (st + 1) * 512)
                        pq = ps512.tile([128, 512], fp32, tag="mm512")
                        for cch in range(NDC):
                            nc.tensor.matmul(
                                pq[:], wq_t[:, cch, :], xT[:, cch, sl],
                                start=(cch == 0), stop=(cch == NDC - 1),
                            )
                        nc.vector.tensor_scalar_add(QT[:, sl], pq[:], bq_t[:])
                    units.append(u_q)

                    def u_k(st=st):
                        sl = slice(st * 512, (st + 1) * 512)
                        pk = ps512.tile([128, 512], fp32, tag="mm512")
                        for cch in range(NDC):
                            nc.tensor.matmul(
                                pk[:], wk_t[:, cch, :], xT[:, cch, sl],
                                start=(cch == 0), stop=(cch == NDC - 1),
                            )
                        nc.vector.tensor_scalar_add(KT[:, sl], pk[:], bk_t[:])
                    units.append(u_k)

                def u_ones():
                    nc.gpsimd.memset(v0[:, :, 64:65], 1.0)
                    nc.gpsimd.memset(v1[:, :, 64:65], 1.0)
                units.append(u_ones)

                for sb in range(NSB):
                    def u_v(sb=sb):
                        pv = ps512.tile([128, 128], fp32, tag="mm512")
                        for cch in range(NDC):
                            nc.tensor.matmul(
                                pv[:], xT[:, cch, sb * 128 : (sb + 1) * 128],
                                wv_t[:, cch, :],
                                start=(cch == 0), stop=(cch == NDC - 1),
                            )
                        nc.vector.tensor_add(
                            v0[:, sb, 0:64], pv[:, 0:64], bv_b[:, 0:64]
                        )
                        nc.vector.tensor_add(
                            v1[:, sb, 0:64], pv[:, 64:128], bv_b[:, 64:128]
                        )
                    units.append(u_v)
                return units

            def B_units(bi):
                """Attention for batch bi, one section per 512-wide q-tile.

                Per kc the two heads' score matmuls are issued back-to-back:
                their stationaries sit in disjoint row groups (partitions
                0-63 / 64-127), so the PE runs them as concurrent row tiles
                fed by one shared QT stream, writing the two halves of one
                [128, 1024] psum pair-tile. One wide exp covers both heads.
                """
                st_ = state[bi]
                QT, KT, v0, v1 = st_["QT"], st_["KT"], st_["v0"], st_["v1"]
                ctx0 = ctx_pool.tile([128, S // 2], cdt, tag="ctx0")
                ctx1 = ctx_pool.tile([128, S // 2], cdt, tag="ctx1")
                st_["ctx"] = (ctx0, ctx1)
                sections = []
                for qt in range(NST):
                    q0 = qt * 512
                    ctxq = (ctx0, ctx1)[qt // 2]
                    qsl_ctx = slice((qt % 2) * 512, (qt % 2 + 1) * 512)
                    units = []
                    avh = []

                    def u_alloc(avh=avh):
                        av0_t = psav.tile([65, 512], fp32, tag="av0")
                        avh.append(av0_t)
                        av1_t = psav.tile([65, 512], fp32, tag="av1")
                        avh.append(av1_t)
                    units.append(u_alloc)

                    exq = []

                    def u_sc(kc, q0=q0, exq=exq):
                        ksl = slice(kc * 128, (kc + 1) * 128)
                        sc = ps1024.tile([128, 1024], fp32, tag="mm1024")
                        nc.tensor.matmul(
                            sc[:, 0:512],
                            KT[0:64, ksl],
                            QT[0:64, q0 : q0 + 512],
                            start=True, stop=True,
                        )
                        nc.tensor.matmul(
                            sc[:, 512:1024],
                            KT[64:128, ksl],
                            QT[64:128, q0 : q0 + 512],
                            start=True, stop=True,
                        )
                        ex = exp_pool.tile([128, 1024], cdt, tag="ex")
                        nc.scalar.activation(ex[:], sc[:], AF.Exp, scale=0.125)
                        exq.append(ex)

                    def u_av(kc, avh=avh, exq=exq):
                        ex = exq[kc]
                        nc.tensor.matmul(
                            avh[0][:], v0[:, kc, :], ex[:, 0:512],
                            start=(kc == 0), stop=(kc == NSB - 1),
                            skip_group_check=True,
                        )
                        nc.tensor.matmul(
                            avh[1][:], v1[:, kc, :], ex[:, 512:1024],
                            start=(kc == 0), stop=(kc == NSB - 1),
                            skip_group_check=True,
                        )

                    for kc in range(NSB):
                        def u_kc(kc=kc, u_sc=u_sc, u_av=u_av):
                            u_sc(kc)
                            if kc > 0:
                                u_av(kc - 1)
                            if kc == NSB - 1:
                                u_av(kc)
                        units.append(u_kc)

                    def u_norm(ctxq=ctxq, qsl_ctx=qsl_ctx, avh=avh):
                        for h in (0, 1):
                            # free the psum bank fast, then normalize
                            av_s = avs_pool.tile([65, 512], fp32, tag="avs")
                            nc.vector.tensor_copy(av_s[:], avh[h][:])
                            # custom-DVE recip needs a partition-0 source;
                            # stage the sumexp row down first
                            se = rec_pool.tile([1, 512], fp32, tag="se")
                            nc.vector.tensor_copy(se[:], av_s[64:65, :])
                            rec = rec_pool.tile([1, 512], fp32, tag="rec")
                            nc.vector.reciprocal_approx_fast(rec[:], se[:])
                            dr = dram_pool.tile([1, 512], fp32, tag="dr")
                            nc.sync.dma_start(dr[:], rec[:])
                            rb = rb_pool.tile([64, 512], fp32, tag="rb")
                            nc.sync.dma_start(rb[:], dr[:].partition_broadcast(64))
                            nc.vector.tensor_mul(
                                ctxq[64 * h : 64 * h + 64, qsl_ctx],
                                av_s[0:64, :], rb[:],
                            )
                    units.append(u_norm)
                    sections.append(units)
                return sections

            def C_units(bi, b):
                st_ = state[bi]
                ctx = st_["ctx"]
                halves = ([], [])
                for sb in range(NSB):
                    def u_o(sb=sb):
                        ctxq = ctx[sb // 8]
                        lsl = slice((sb % 8) * 128, (sb % 8 + 1) * 128)
                        ot = out_pool.tile([128, D], fp32, tag="ot")
                        for half in range(2):
                            osl = slice(half * 512, (half + 1) * 512)
                            po = ps512.tile([128, 512], fp32, tag="mm512")
                            nc.tensor.matmul(
                                po[:], ctxq[:, lsl], wo_t[:, osl],
                                start=True, stop=True,
                            )
                            nc.vector.tensor_copy(ot[:, osl], po[:])
                        nc.sync.dma_start(
                            out_d[b, sb * 128 : (sb + 1) * 128, :], ot[:]
                        )
                    halves[sb // 8].append(u_o)
                return halves

            bs = [bb for _ in range(repeat) for bb in range(B)]
            n_x_units = NDC if dtype == "bf16" else NSB
            a0 = A_units(0, bs[0])
            for u in a0[:n_x_units]:
                u()
            load_consts()
            for u in a0[n_x_units:]:
                u()
            c_tail = []
            for bi, b in enumerate(bs):
                secs = B_units(bi)
                if bi + 1 < len(bs):
                    a_next = A_units(bi + 1, bs[bi + 1])
                    xdmas, a_rest = a_next[:n_x_units], a_next[n_x_units:]
                else:
                    xdmas, a_rest = [], []
                # start next batch's x transposes immediately
                for u in xdmas:
                    u()
                half = len(a_rest) // 2
                # sec0 gets projection fill only: the previous batch's last
                # q-tile norm chain (recip + broadcast bounce) is still in
                # flight, so its out-proj (c_tail) starts in sec1.
                for u in _interleave(secs[0], a_rest[:half]):
                    u()
                for u in _interleave(secs[1], c_tail, start_frac=0.0):
                    u()
                c_head, new_tail = C_units(bi, b)
                for u in _interleave(secs[2], a_rest[half:]):
                    u()
                for u in _interleave(secs[3], c_head, start_frac=0.1):
                    u()
                c_tail = new_tail
                del state[bi]
            for u in c_tail:
                u()

    nc.compile()
    return nc


def _get_nc(repeat=1, bench_io=False, dtype=None):
    if dtype is None:
        dtype = DTYPE
    key = f"nc{repeat}_{bench_io}_{dtype}"
    if key not in _CACHE:
        _CACHE[key] = _build(repeat, bench_io, dtype)
    return _CACHE[key]


def kernel(**inputs):
    global LAST_RESULTS
    import ml_dtypes
    from concourse.bass_utils import run_bass_kernel_spmd

    cdt = ml_dtypes.bfloat16 if DTYPE == "bf16" else np.float32
    x = np.ascontiguousarray(np.asarray(inputs["x"], dtype=np.float32).astype(cdt))
    Wq = np.asarray(inputs["Wq"], dtype=np.float32).astype(cdt)
    Wk = np.asarray(inputs["Wk"], dtype=np.float32).astype(cdt)
    Wv = np.asarray(inputs["Wv"], dtype=np.float32).astype(cdt)
    Wo = np.asarray(inputs["Wo"], dtype=np.float32).astype(cdt)
    bq = np.asarray(inputs["bq"], dtype=np.float32)
    bk = np.asarray(inputs["bk"], dtype=np.float32)
    bv = np.asarray(inputs["bv"], dtype=np.float32)
    bo = np.asarray(inputs["bo"], dtype=np.float32)

    nc = _get_nc()
    in_maps = []
    for c in range(NCORES):
        cs = slice(CS * c, CS * (c + 1))
        in_maps.append(
            {
                "x": x,
                "wq": np.ascontiguousarray(Wq[:, cs]),
                "wk": np.ascontiguousarray(Wk[:, cs]),
                "wv": np.ascontiguousarray(Wv[:, cs]),
                "wo": np.ascontiguousarray(Wo[cs, :]),
                "bq": np.ascontiguousarray(bq[cs]),
                "bk": np.ascontiguousarray(bk[cs]),
                "bv": np.ascontiguousarray(bv[cs]),
            }
        )
    res = run_bass_kernel_spmd(
        nc, in_maps, core_ids=list(range(NCORES)), trace=TRACE
    )
    LAST_RESULTS = res
    acc = np.zeros((B, S, D), dtype=np.float64)
    for c in range(NCORES):
        acc += res.results[c]["out"]
    acc += bo
    return acc.astype(np.float32)



# revision 15
# speedup vs baseline: 1.4141x; 1.0616x over previous
"""Multi-head attention on 8 Trainium2 NeuronCores (tensor-parallel over heads).

B=4, S=2048, D=1024, H=16 heads of DK=64. Each core owns 2 heads (a
128-channel slice of the QKV projections). Per core, per batch b:
  xT   = transpose(x[b])           [d=128 x 8, S]  (DMA transpose, bf16)
  QT   = (Wq_c)^T x^T + bq_c       [128, S]        (channels on partitions)
  KT   = (Wk_c)^T x^T + bk_c       [128, S]
  V    = x Wv_c + bv_c             [S, 128] stored per-head with a ones col
  per q-tile (512 q cols), accumulate over k-chunks kc:
    scT = K Q^T            [k=128, 2x512] psum; both heads issued
                           back-to-back as concurrent PE row tiles
                           (rows 0-63 / 64-127) sharing one QT stream
    ex  = exp(scT / 8)     bf16 (one wide activation, both heads)
    av_h += V_aug^T ex_h   [65, 512] per head; rows 0-63 ctx^T, row 64 sumexp
  ctxT = av[0:64] * recip(av[64])  (recip on the 64-row broadcast)
  out[b] partial = ctx^T Wo_c      [S, D] fp32  (host sums partials + bo)

Matmul inputs are bf16 (1 cyc/col on PE); accumulation is fp32 in PSUM;
softmax stats and the output are fp32. The emission is software-pipelined:
batch b's attention (ACT-heavy) is interleaved with batch b+1's projections
and batch b-1's output projection (PE/DMA-heavy) so all engines stay fed.
"""

import numpy as np

B, S, D, H, DK = 4, 2048, 1024, 16, 64
NCORES = 8
CS = D // NCORES  # 128 channels (2 heads) per core
NSB = S // 128    # 16 s-blocks
NST = S // 512    # 4 s-tiles
NDC = D // 128    # 8 d-chunks

DTYPE = "bf16"  # "bf16" | "fp32"
TRACE = False
LAST_RESULTS = None
_CACHE = {}


def _interleave(main, fill, start_frac=0.2):
    """Spread fill units evenly between main units (order preserved).
    No fill before start_frac of main has been emitted: the engines run
    in static order, so a fill unit whose inputs aren't ready yet would
    stall them."""
    out = []
    fi = 0
    n0 = int(len(main) * start_frac)
    for i, u in enumerate(main):
        out.append(u)
        if i < n0:
            continue
        want = (i - n0 + 1) * len(fill) // max(1, len(main) - n0)
        while fi < want:
            out.append(fill[fi])
            fi += 1
    out.extend(fill[fi:])
    return out


def _build(repeat=1, bench_io=False, dtype=DTYPE):
    import concourse.bass as bass  # noqa: F401
    import concourse.mybir as mybir
    import concourse.tile as tile
    from concourse import bacc
    from concourse.masks import make_identity

    fp32 = mybir.dt.float32
    fp16 = mybir.dt.float16
    fp8d = mybir.dt.float8e4
    DRMODE = mybir.MatmulPerfMode.DoubleRow
    cdt = mybir.dt.bfloat16 if dtype == "bf16" else fp32
    AF = mybir.ActivationFunctionType

    nc = bacc.Bacc(None, target_bir_lowering=False)
    if bench_io:
        # timing variant: big tensors stay on-device (garbage contents), tiny
        # external I/O so per-call tunnel transfers don't mask exec time
        x_d = nc.dram_tensor("xint", [B, S, D], cdt)
        out_d = nc.dram_tensor("outint", [B, S, D], fp32)
        xin_d = nc.declare_dram_parameter("xin", [128, 128], fp32, isOutput=False)
        xout_d = nc.declare_dram_parameter("xout", [128, 128], fp32, isOutput=True)
    else:
        x_d = nc.declare_dram_parameter("x", [B, S, D], cdt, isOutput=False)
        out_d = nc.declare_dram_parameter("out", [B, S, D], fp32, isOutput=True)
    wq_d = nc.declare_dram_parameter("wq", [D, CS], cdt, isOutput=False)
    wk_d = nc.declare_dram_parameter("wk", [D, CS], cdt, isOutput=False)
    wv_d = nc.declare_dram_parameter("wv", [D, CS], cdt, isOutput=False)
    wo_d = nc.declare_dram_parameter("wo", [CS, D], cdt, isOutput=False)
    bq_d = nc.declare_dram_parameter("bq", [CS], fp32, isOutput=False)
    bk_d = nc.declare_dram_parameter("bk", [CS], fp32, isOutput=False)
    bv_d = nc.declare_dram_parameter("bv", [CS], fp32, isOutput=False)

    with tile.TileContext(nc) as tc:
        with (
            tc.tile_pool(name="consts", bufs=1) as consts,
            tc.tile_pool(name="xt", bufs=2) as xt_pool,
            tc.tile_pool(name="xload", bufs=3) as xload,
            tc.tile_pool(name="qk", bufs=2) as qk_pool,
            tc.tile_pool(name="vp", bufs=2) as v_pool,
            tc.tile_pool(name="exp", bufs=6) as exp_pool,
            tc.tile_pool(name="ctx", bufs=2) as ctx_pool,
            tc.tile_pool(name="avs", bufs=6) as avs_pool,
            tc.tile_pool(name="rec", bufs=4) as rec_pool,
            tc.tile_pool(name="rb", bufs=4) as rb_pool,
            tc.tile_pool(name="outp", bufs=4) as out_pool,
            tc.tile_pool(name="drp", bufs=8, space="DRAM") as dram_pool,
            tc.tile_pool(name="ps1024", bufs=2, space="PSUM") as ps1024,
            tc.tile_pool(name="ps512", bufs=2, space="PSUM") as ps512,
            tc.tile_pool(name="psav", bufs=1, space="PSUM") as psav,
        ):
            # ---- constants (tiles now, loads deferred until after the
            # first x-transpose DMAs are queued) ----
            wq_t = consts.tile([128, NDC, CS], cdt, tag="wq")
            wk_t = consts.tile([128, NDC, CS], cdt, tag="wk")
            wv_t = consts.tile([128, NDC, CS], cdt, tag="wv")
            wo_t = consts.tile([128, D], cdt, tag="wo")
            bq_t = consts.tile([128, 1], fp32, tag="bq")
            bk_t = consts.tile([128, 1], fp32, tag="bk")
            ones_r = consts.tile([128, 64], fp32, tag="ones_r")
            bv_b = consts.tile([128, CS], fp32, tag="bvb")
            if dtype != "bf16":
                ident = consts.tile([128, 128], cdt, tag="ident")

            def load_consts():
                nc.sync.dma_start(
                    wq_t[:], wq_d[:].rearrange("(c p) m -> p c m", p=128)
                )
                nc.sync.dma_start(
                    wk_t[:], wk_d[:].rearrange("(c p) m -> p c m", p=128)
                )
                nc.sync.dma_start(
                    wv_t[:], wv_d[:].rearrange("(c p) m -> p c m", p=128)
                )
                nc.sync.dma_start(wo_t[:], wo_d[:])
                nc.sync.dma_start(bq_t[:], bq_d[:].rearrange("(p o) -> p o", o=1))
                nc.sync.dma_start(bk_t[:], bk_d[:].rearrange("(p o) -> p o", o=1))
                nc.gpsimd.memset(ones_r[:], 1.0)
                nc.sync.dma_start(
                    bv_b[:],
                    bv_d[:].rearrange("(o f) -> o f", o=1).partition_broadcast(128),
                )
                if dtype != "bf16":
                    make_identity(nc, ident[:])
                if bench_io:
                    tio = consts.tile([128, 128], fp32, tag="tio")
                    nc.sync.dma_start(tio[:], xin_d[:])
                    nc.sync.dma_start(xout_d[:], tio[:])

            state = {}

            def A_units(bi, b):
                """x transpose + QKV projections for batch index bi."""
                xT = xt_pool.tile([128, NDC, S], cdt, tag="xT")
                QT = qk_pool.tile([128, S], cdt, tag="QT")
                KT = qk_pool.tile([128, S], cdt, tag="KT")
                v0 = v_pool.tile([128, NSB // 2, 2, 80], fp8d, tag="v0")
                v1 = v_pool.tile([128, NSB // 2, 2, 80], fp8d, tag="v1")
                state[bi] = dict(xT=xT, QT=QT, KT=KT, v0=v0, v1=v1)
                units = []
                if dtype == "bf16":
                    xr = x_d[b].rearrange("M (c p) -> M c p", p=128)
                    for cch in range(NDC):
                        units.append(
                            lambda cch=cch: nc.sync.dma_start(
                                xT[:, cch, :], xr[:, cch], transpose=True
                            )
                        )
                else:
                    for sb in range(NSB):
                        def u_x(sb=sb):
                            xl = xload.tile([128, D], cdt, tag="xl")
                            nc.sync.dma_start(
                                xl[:], x_d[b, sb * 128 : (sb + 1) * 128, :]
                            )
                            for cch in range(NDC):
                                pt = ps512.tile([128, 128], fp32, tag="mm512")
                                nc.tensor.transpose(
                                    pt[:], xl[:, cch * 128 : (cch + 1) * 128],
                                    ident[:],
                                )
                                nc.vector.tensor_copy(
                                    xT[:, cch, sb * 128 : (sb + 1) * 128], pt[:]
                                )
                        units.append(u_x)

                for st in range(NST):
                    def u_q(st=st):
                        sl = slice(st * 512, (st + 1) * 512)
                        pq = ps512.tile([128, 512], fp32, tag="mm512")
                        for cch in range(NDC):
                            nc.tensor.matmul(
                                pq[:], wq_t[:, cch, :], xT[:, cch, sl],
                                start=(cch == 0), stop=(cch == NDC - 1),
                            )
                        nc.vector.tensor_scalar_add(QT[:, sl], pq[:], bq_t[:])
                    units.append(u_q)

                    def u_k(st=st):
                        sl = slice(st * 512, (st + 1) * 512)
                        pk = ps512.tile([128, 512], fp32, tag="mm512")
                        for cch in range(NDC):
                            nc.tensor.matmul(
                                pk[:], wk_t[:, cch, :], xT[:, cch, sl],
                                start=(cch == 0), stop=(cch == NDC - 1),
                            )
                        nc.vector.tensor_scalar_add(KT[:, sl], pk[:], bk_t[:])
                    units.append(u_k)

                def u_ones():
                    nc.gpsimd.memset(v0[:, :, :, 64:65], 1.0)
                    nc.gpsimd.memset(v1[:, :, :, 64:65], 1.0)
                units.append(u_ones)

                for sb in range(NSB):
                    def u_v(sb=sb):
                        pv = ps512.tile([128, 128], fp32, tag="mm512")
                        for cch in range(NDC):
                            nc.tensor.matmul(
                                pv[:], xT[:, cch, sb * 128 : (sb + 1) * 128],
                                wv_t[:, cch, :],
                                start=(cch == 0), stop=(cch == NDC - 1),
                            )
                        nc.vector.tensor_add(
                            v0[:, sb // 2, sb % 2, 0:64], pv[:, 0:64],
                            bv_b[:, 0:64]
                        )
                        nc.vector.tensor_add(
                            v1[:, sb // 2, sb % 2, 0:64], pv[:, 64:128],
                            bv_b[:, 64:128]
                        )
                    units.append(u_v)
                return units

            def B_units(bi):
                """Attention for batch bi, one section per 512-wide q-tile.

                Per kc the two heads' score matmuls are issued back-to-back:
                their stationaries sit in disjoint row groups (partitions
                0-63 / 64-127), so the PE runs them as concurrent row tiles
                fed by one shared QT stream, writing the two halves of one
                [128, 1024] psum pair-tile. One wide exp covers both heads.
                """
                st_ = state[bi]
                QT, KT, v0, v1 = st_["QT"], st_["KT"], st_["v0"], st_["v1"]
                ctx0 = ctx_pool.tile([128, S // 2], cdt, tag="ctx0")
                ctx1 = ctx_pool.tile([128, S // 2], cdt, tag="ctx1")
                st_["ctx"] = (ctx0, ctx1)
                sections = []
                for qt in range(NST):
                    q0 = qt * 512
                    ctxq = (ctx0, ctx1)[qt // 2]
                    qsl_ctx = slice((qt % 2) * 512, (qt % 2 + 1) * 512)
                    units = []
                    avh = []

                    def u_alloc(avh=avh):
                        av0_t = psav.tile([80, 512], fp32, tag="av0")
                        avh.append(av0_t)
                        av1_t = psav.tile([80, 512], fp32, tag="av1")
                        avh.append(av1_t)
                    units.append(u_alloc)

                    exq = []

                    def u_sc(kc, q0=q0, exq=exq):
                        ksl = slice(kc * 128, (kc + 1) * 128)
                        sc = ps1024.tile([128, 1024], fp32, tag="mm1024")
                        nc.tensor.matmul(
                            sc[:, 0:512],
                            KT[0:64, ksl],
                            QT[0:64, q0 : q0 + 512],
                            start=True, stop=True,
                        )
                        nc.tensor.matmul(
                            sc[:, 512:1024],
                            KT[64:128, ksl],
                            QT[64:128, q0 : q0 + 512],
                            start=True, stop=True,
                        )
                        if kc % 2 == 0:
                            exp_t = exp_pool.tile([128, 2, 1024], fp8d, tag="ex")
                            exq.append(exp_t)
                        nc.scalar.activation(
                            exq[kc // 2][:, kc % 2, :], sc[:], AF.Exp,
                            scale=0.125,
                        )

                    NKCP = NSB // 2

                    def u_av(kcp, avh=avh, exq=exq):
                        ex = exq[kcp]
                        nc.tensor.matmul(
                            avh[0][:], v0[:, kcp], ex[:, :, 0:512],
                            start=(kcp == 0), stop=(kcp == NKCP - 1),
                            perf_mode=DRMODE, skip_group_check=True,
                        )
                        nc.tensor.matmul(
                            avh[1][:], v1[:, kcp], ex[:, :, 512:1024],
                            start=(kcp == 0), stop=(kcp == NKCP - 1),
                            perf_mode=DRMODE, skip_group_check=True,
                        )

                    for kc in range(NSB):
                        def u_kc(kc=kc, u_sc=u_sc, u_av=u_av):
                            u_sc(kc)
                            if kc % 2 == 0 and kc > 0:
                                u_av(kc // 2 - 1)
                            if kc == NSB - 1:
                                u_av(NKCP - 1)
                        units.append(u_kc)

                    def u_norm(ctxq=ctxq, qsl_ctx=qsl_ctx, avh=avh):
                        for h in (0, 1):
                            # free the psum bank fast, then normalize
                            av_s = avs_pool.tile([65, 512], fp32, tag="avs")
                            nc.vector.tensor_copy(av_s[:], avh[h][0:65, :])
                            # custom-DVE recip needs a partition-0 source;
                            # stage the sumexp row down first
                            se = rec_pool.tile([1, 512], fp32, tag="se")
                            nc.vector.tensor_copy(se[:], av_s[64:65, :])
                            rec = rec_pool.tile([1, 512], fp32, tag="rec")
                            nc.vector.reciprocal_approx_fast(rec[:], se[:])
                            dr = dram_pool.tile([1, 512], fp32, tag="dr")
                            nc.sync.dma_start(dr[:], rec[:])
                            rb = rb_pool.tile([64, 512], fp32, tag="rb")
                            nc.sync.dma_start(rb[:], dr[:].partition_broadcast(64))
                            nc.vector.tensor_mul(
                                ctxq[64 * h : 64 * h + 64, qsl_ctx],
                                av_s[0:64, :], rb[:],
                            )
                    units.append(u_norm)
                    sections.append(units)
                return sections

            def C_units(bi, b):
                st_ = state[bi]
                ctx = st_["ctx"]
                halves = ([], [])
                for sb in range(NSB):
                    def u_o(sb=sb):
                        ctxq = ctx[sb // 8]
                        lsl = slice((sb % 8) * 128, (sb % 8 + 1) * 128)
                        ot = out_pool.tile([128, D], fp32, tag="ot")
                        for half in range(2):
                            osl = slice(half * 512, (half + 1) * 512)
                            po = ps512.tile([128, 512], fp32, tag="mm512")
                            nc.tensor.matmul(
                                po[:], ctxq[:, lsl], wo_t[:, osl],
                                start=True, stop=True,
                            )
                            nc.vector.tensor_copy(ot[:, osl], po[:])
                        nc.sync.dma_start(
                            out_d[b, sb * 128 : (sb + 1) * 128, :], ot[:]
                        )
                    halves[sb // 8].append(u_o)
                return halves

            bs = [bb for _ in range(repeat) for bb in range(B)]
            n_x_units = NDC if dtype == "bf16" else NSB
            a0 = A_units(0, bs[0])
            for u in a0[:n_x_units]:
                u()
            load_consts()
            for u in a0[n_x_units:]:
                u()
            c_tail = []
            for bi, b in enumerate(bs):
                secs = B_units(bi)
                if bi + 1 < len(bs):
                    a_next = A_units(bi + 1, bs[bi + 1])
                    xdmas, a_rest = a_next[:n_x_units], a_next[n_x_units:]
                else:
                    xdmas, a_rest = [], []
                # start next batch's x transposes immediately
                for u in xdmas:
                    u()
                half = len(a_rest) // 2
                # sec0 gets projection fill only: the previous batch's last
                # q-tile norm chain (recip + broadcast bounce) is still in
                # flight, so its out-proj (c_tail) starts in sec1.
                for u in _interleave(secs[0], a_rest[:half]):
                    u()
                for u in _interleave(secs[1], c_tail, start_frac=0.0):
                    u()
                c_head, new_tail = C_units(bi, b)
                for u in _interleave(secs[2], a_rest[half:]):
                    u()
                for u in _interleave(secs[3], c_head, start_frac=0.1):
                    u()
                c_tail = new_tail
                del state[bi]
            for u in c_tail:
                u()

    nc.compile()
    return nc


def _get_nc(repeat=1, bench_io=False, dtype=None):
    if dtype is None:
        dtype = DTYPE
    key = f"nc{repeat}_{bench_io}_{dtype}"
    if key not in _CACHE:
        _CACHE[key] = _build(repeat, bench_io, dtype)
    return _CACHE[key]


def kernel(**inputs):
    global LAST_RESULTS
    import ml_dtypes
    from concourse.bass_utils import run_bass_kernel_spmd

    cdt = ml_dtypes.bfloat16 if DTYPE == "bf16" else np.float32
    x = np.ascontiguousarray(np.asarray(inputs["x"], dtype=np.float32).astype(cdt))
    Wq = np.asarray(inputs["Wq"], dtype=np.float32).astype(cdt)
    Wk = np.asarray(inputs["Wk"], dtype=np.float32).astype(cdt)
    Wv = np.asarray(inputs["Wv"], dtype=np.float32).astype(cdt)
    Wo = np.asarray(inputs["Wo"], dtype=np.float32).astype(cdt)
    bq = np.asarray(inputs["bq"], dtype=np.float32)
    bk = np.asarray(inputs["bk"], dtype=np.float32)
    bv = np.asarray(inputs["bv"], dtype=np.float32)
    bo = np.asarray(inputs["bo"], dtype=np.float32)

    nc = _get_nc()
    in_maps = []
    for c in range(NCORES):
        cs = slice(CS * c, CS * (c + 1))
        in_maps.append(
            {
                "x": x,
                "wq": np.ascontiguousarray(Wq[:, cs]),
                "wk": np.ascontiguousarray(Wk[:, cs]),
                "wv": np.ascontiguousarray(Wv[:, cs]),
                "wo": np.ascontiguousarray(Wo[cs, :]),
                "bq": np.ascontiguousarray(bq[cs]),
                "bk": np.ascontiguousarray(bk[cs]),
                "bv": np.ascontiguousarray(bv[cs]),
            }
        )
    res = run_bass_kernel_spmd(
        nc, in_maps, core_ids=list(range(NCORES)), trace=TRACE
    )
    LAST_RESULTS = res
    acc = np.zeros((B, S, D), dtype=np.float64)
    for c in range(NCORES):
        acc += res.results[c]["out"]
    acc += bo
    return acc.astype(np.float32)

